# revision 13
# baseline (speedup 1.0000x reference)
"""Trainium2 Bass kernel for BCNet-style bilinear head.

Computes logits[b,h,n,d] = sum_k hm[h,k] * v_[b,n,k] * q_[b,d,k] + h_bias
where v_ = v @ wn(Wv,gv).T + bv,  q_ = q @ wn(Wq,gq).T + bq,
wn(W,g) = (g/||W||_F) * W.

Restructured to minimize FLOPs (150 GF total instead of naive 219 GF):
  per batch b:
    q_T[k,d]    = (sq*Wq) @ q[b].T + bq           (contract C)
    qh[k,h*D+d] = hm[h,k] * q_T[k,d]
    GT[cv,hd]   = sum_k (sv*Wv)[k,cv] * qh[k,hd]  (contract K)
    bvrow[hd]   = sum_k bv[k]*hm[h,k]*q_T[k,d] (+h_bias)
    out[b][n,hd] = sum_cv vT[cv,n] * GT[cv,hd] + bvrow  (ones-row matmul pass)
Sharding: data-parallel over B=16 across 8 cores (2 batches/core).
All matmuls bf16 with fp32 PSUM accumulation (measured rel err ~3.5e-3).

Both batches' phase-1 runs inside the initial weight-DMA window; batch 1's
qh tiles reuse the wq SBUF slots (same shape) once wq is consumed.
"""

import sys

for _p in ("/opt/trn_rl_repo",):
    if _p not in sys.path:
        sys.path.insert(0, _p)

import numpy as np
import ml_dtypes

from concourse import bass, bacc, tile, mybir
from concourse.bass_utils import run_bass_kernel_spmd

BF16 = ml_dtypes.bfloat16
F32 = mybir.dt.float32
BF = mybir.dt.bfloat16
AF = mybir.ActivationFunctionType

B, N, C, D, K, H = 16, 1024, 1024, 128, 3072, 8
KT, CT, NT = K // 128, C // 128, N // 128  # 24, 8, 8
HD = H * D  # 1024
NCORES = 8
BPC = B // NCORES  # batches per core

_CACHE = {}


def _build_program():
    nc = bacc.Bacc("TRN2", target_bir_lowering=False, debug=False,
                   num_devices=NCORES)

    vT_d = nc.dram_tensor("vT", [BPC, C, N], BF, kind="ExternalInput")
    # qt2[b, p, ct*128+d] = q[b, d, ct*128+p]
    qT_d = nc.dram_tensor("qT", [BPC, 128, CT * D], BF, kind="ExternalInput")
    # wq2[kt, p, ct*128+j] = (sq*Wq)[kt*128+j, ct*128+p]
    wqT_d = nc.dram_tensor("wqT", [KT, 128, CT * 128], BF, kind="ExternalInput")
    wv_d = nc.dram_tensor("wv", [K, C], BF, kind="ExternalInput")
    # bvhm[p, kt*8+h] = bv[k]*hm[h,k], k = kt*128+p
    bvhm_d = nc.dram_tensor("bvhm", [128, KT * H], BF, kind="ExternalInput")
    # cst columns: [0:KT] bq_t, [KT:KT+KT*H] hm_t, [216] h_bias col
    XBQ, XHM, XHB = 0, KT, KT + KT * H
    XTOT = KT + KT * H + 1
    cst_d = nc.dram_tensor("cst", [128, XTOT], F32, kind="ExternalInput")
    out_d = nc.dram_tensor("out", [BPC, H, N, D], F32, kind="ExternalOutput")

    with tile.TileContext(nc) as tc:
        with (
            tc.tile_pool(name="wq", bufs=1) as p_wq,
            tc.tile_pool(name="wv", bufs=1) as p_wv,
            tc.tile_pool(name="qh", bufs=1) as p_qh,
            tc.tile_pool(name="vt", bufs=1) as p_vt,
            tc.tile_pool(name="gt", bufs=1) as p_gt,
            tc.tile_pool(name="qsb", bufs=2) as p_qsb,
            tc.tile_pool(name="qt", bufs=2) as p_qt,
            tc.tile_pool(name="small", bufs=1) as p_small,
            tc.tile_pool(name="bvr", bufs=2) as p_bvr,
            tc.tile_pool(name="osb", bufs=3) as p_osb,
            tc.tile_pool(name="psq", bufs=2, space="PSUM") as ps_q,
            tc.tile_pool(name="psbv", bufs=2, space="PSUM") as ps_bv,
            tc.tile_pool(name="psg", bufs=2, space="PSUM") as ps_g,
            tc.tile_pool(name="pso", bufs=2, space="PSUM") as ps_o,
        ):
            cst_sb = p_small.tile([128, XTOT], F32)
            nc.sync.dma_start(cst_sb[:], cst_d.ap())
            bvhm_sb = p_small.tile([128, KT * H], BF)
            nc.sync.dma_start(bvhm_sb[:], bvhm_d.ap())
            ones_sb = p_small.tile([1, 128], BF)
            nc.vector.memset(ones_sb[:], 1.0)

            qt_sb = []
            for b in range(BPC):
                t = p_qt.tile([128, CT * D], BF)
                nc.sync.dma_start(t[:], qT_d.ap()[b])
                qt_sb.append(t)

            # wq per k-tile: phase-1 consumes slices as they stream in
            wq_sb = []
            for kt in range(KT):
                t = p_wq.tile([128, CT * 128], BF, tag=f"wq{kt}")
                nc.sync.dma_start(t[:], wqT_d.ap()[kt])
                wq_sb.append(t)

            # ---- phase 1 (both batches): q_T, qh, bvterm ----
            q_sb = []
            qh_tiles = [[], []]
            for b in range(BPC):
                qs = p_qsb.tile([128, KT * D], BF)
                q_sb.append(qs)
                for kt in range(KT):
                    pq = ps_q.tile([128, D], F32)
                    for ct in range(CT):
                        nc.tensor.matmul(
                            pq[:],
                            wq_sb[kt][:, ct * 128:(ct + 1) * 128],
                            qt_sb[b][:, ct * D:(ct + 1) * D],
                            start=(ct == 0), stop=(ct == CT - 1))
                    nc.scalar.activation(
                        qs[:, kt * D:(kt + 1) * D], pq[:], AF.Identity,
                        bias=cst_sb[:, XBQ + kt:XBQ + kt + 1], scale=1.0)
                    # qh[k, h*D:(h+1)*D] = hm[h,k] * q_T[k, :]
                    if b == 0:
                        qh = p_qh.tile([128, HD], BF, tag=f"qh{kt}")
                    else:
                        qh = p_wq.tile([128, HD], BF, tag=f"wq{kt}")
                    qh_tiles[b].append(qh)
                    for h in range(H):
                        nc.vector.tensor_scalar_mul(
                            qh[:, h * D:(h + 1) * D],
                            qs[:, kt * D:(kt + 1) * D],
                            cst_sb[:, XHM + kt * H + h: XHM + kt * H + h + 1])

            # bvterm[h,d] = sum_k bv[k]*hm[h,k]*q_T[k,d]; then to [1, HD] row
            bvrow = []
            for b in range(BPC):
                pbv = ps_bv.tile([8, D], F32)
                for kt in range(KT):
                    nc.tensor.matmul(
                        pbv[:],
                        bvhm_sb[:, kt * H:(kt + 1) * H],
                        q_sb[b][:, kt * D:(kt + 1) * D],
                        start=(kt == 0), stop=(kt == KT - 1))
                bvsb = p_bvr.tile([8, D], BF, tag="bvsb")
                nc.scalar.activation(bvsb[:], pbv[:], AF.Identity,
                                     bias=cst_sb[0:8, XHB:XHB + 1], scale=1.0)
                row = p_bvr.tile([1, HD], BF, tag="bvrow")
                bvrow.append(row)
                for h in range(H):
                    nc.sync.dma_start(row[0:1, h * D:(h + 1) * D],
                                      bvsb[h:h + 1, :])

            # ---- deferred big loads ----
            wv_sb = p_wv.tile([128, KT * C], BF)
            nc.sync.dma_start(
                wv_sb[:].rearrange("p (kt c) -> p kt c", kt=KT),
                wv_d.ap().rearrange("(kt p) c -> p kt c", p=128))

            for b in range(BPC):
                vt_sb = p_vt.tile([128, CT * N], BF)
                nc.sync.dma_start(
                    vt_sb[:].rearrange("p (ct n) -> p ct n", ct=CT),
                    vT_d.ap()[b].rearrange("(ct p) n -> p ct n", p=128))

                # ---- G: GT[cv, hd] = sum_k wv[k,cv] * qh[k,hd] ----
                gt_sb = p_gt.tile([128, CT * HD], BF)
                for ct in range(CT):
                    for c2 in range(2):
                        pg = ps_g.tile([128, 512], F32)
                        for kt in range(KT):
                            nc.tensor.matmul(
                                pg[:],
                                wv_sb[:, kt * C + ct * 128: kt * C + (ct + 1) * 128],
                                qh_tiles[b][kt][:, c2 * 512:(c2 + 1) * 512],
                                start=(kt == 0), stop=(kt == KT - 1))
                        nc.scalar.activation(
                            gt_sb[:, ct * HD + c2 * 512: ct * HD + c2 * 512 + 512],
                            pg[:], AF.Copy)

                # ---- final: out[n,hd] = sum_cv vT[cv,n]*GT[cv,hd] + bvrow ----
                for nt in range(NT):
                    for c2 in range(2):
                        po = ps_o.tile([128, 512], F32)
                        nc.tensor.matmul(
                            po[:], ones_sb[0:1, :],
                            bvrow[b][0:1, c2 * 512:(c2 + 1) * 512],
                            start=True, stop=False)
                        for ct in range(CT):
                            nc.tensor.matmul(
                                po[:],
                                vt_sb[:, ct * N + nt * 128: ct * N + (nt + 1) * 128],
                                gt_sb[:, ct * HD + c2 * 512: ct * HD + c2 * 512 + 512],
                                start=False, stop=(ct == CT - 1))
                        ob = p_osb.tile([128, 512], F32)
                        nc.vector.tensor_copy(ob[:], po[:])
                        nc.sync.dma_start(
                            out_d.ap()[b, c2 * 4:(c2 + 1) * 4,
                                       nt * 128:(nt + 1) * 128, :]
                            .rearrange("h n d -> n h d"),
                            ob[:].rearrange("n (h d) -> n h d", h=4))

    nc.compile()
    return nc


def _get_program():
    if "nc" not in _CACHE:
        _CACHE["nc"] = _build_program()
    return _CACHE["nc"]


def _prep_inputs(v, q, Wv, gv, bv, Wq, gq, bq, h_mat, h_bias):
    v = np.asarray(v, np.float32)
    q = np.asarray(q, np.float32)
    Wv = np.asarray(Wv, np.float32)
    Wq = np.asarray(Wq, np.float32)
    bv = np.asarray(bv, np.float32)
    bq = np.asarray(bq, np.float32)
    sv = np.float32(gv) / np.float32(np.linalg.norm(Wv))
    sq = np.float32(gq) / np.float32(np.linalg.norm(Wq))
    hm = np.asarray(h_mat, np.float32)[0, :, 0, :]  # (H, K)
    hb = np.asarray(h_bias, np.float32).reshape(H)

    wv_b = (Wv * sv).astype(BF16)                                     # (K, C)
    wqT_b = np.ascontiguousarray(
        (Wq * sq).reshape(KT, 128, CT, 128)
        .transpose(0, 3, 2, 1).reshape(KT, 128, CT * 128)).astype(BF16)
    vT = np.ascontiguousarray(v.transpose(0, 2, 1)).astype(BF16)      # (B,C,N)
    qT = np.ascontiguousarray(
        q.reshape(B, D, CT, 128).transpose(0, 3, 2, 1)
        .reshape(B, 128, CT * D)).astype(BF16)

    bq_t = np.ascontiguousarray(bq.reshape(KT, 128).T)                # (128,KT)
    hm_t = np.ascontiguousarray(
        hm.T.reshape(KT, 128, H).transpose(1, 0, 2).reshape(128, KT * H))
    hb_col = np.zeros((128, 1), np.float32)
    hb_col[:H, 0] = hb
    cst = np.concatenate([bq_t, hm_t, hb_col], axis=1).astype(np.float32)
    # bvhm[p, kt*8+h] = bv[k]*hm[h,k], k = kt*128+p
    bvhm = np.ascontiguousarray(
        (bv[None, :] * hm).T.reshape(KT, 128, H)
        .transpose(1, 0, 2).reshape(128, KT * H)).astype(BF16)

    in_maps = []
    for core in range(NCORES):
        b0 = core * BPC
        in_maps.append({
            "vT": np.ascontiguousarray(vT[b0:b0 + BPC]),
            "qT": np.ascontiguousarray(qT[b0:b0 + BPC]),
            "wqT": wqT_b,
            "wv": wv_b,
            "bvhm": bvhm,
            "cst": cst,
        })
    return in_maps


def run_device(in_maps, **kw):
    nc = _get_program()
    return run_bass_kernel_spmd(nc, in_maps, list(range(NCORES)), **kw)


def kernel(v, q, Wv, gv, bv, Wq, gq, bq, h_mat, h_bias):
    in_maps = _prep_inputs(v, q, Wv, gv, bv, Wq, gq, bq, h_mat, h_bias)
    res = run_device(in_maps)
    out = np.empty((B, H, N, D), np.float32)
    for core in range(NCORES):
        b0 = core * BPC
        out[b0:b0 + BPC] = res.results[core]["out"]
    return out


if __name__ == "__main__":
    rng = np.random.default_rng(0)
    ins = {
        "v": rng.standard_normal((B, N, C), np.float32),
        "q": rng.standard_normal((B, D, C), np.float32),
        "Wv": rng.standard_normal((K, C), np.float32) * 0.02,
        "gv": np.ones((), np.float32),
        "bv": rng.standard_normal((K,), np.float32) * 0.02,
        "Wq": rng.standard_normal((K, C), np.float32) * 0.02,
        "gq": np.ones((), np.float32),
        "bq": rng.standard_normal((K,), np.float32) * 0.02,
        "h_mat": rng.standard_normal((1, H, 1, K), np.float32) * 0.02,
        "h_bias": np.zeros((1, H, 1, 1), np.float32),
    }
    out = kernel(**ins)
    print("out", out.shape, out.dtype, np.abs(out).max())
